# revision 14
# baseline (speedup 1.0000x reference)
"""Distributed Trainium2 Bass kernel for causal multi-head attention with RoPE.

Reference computation (B=2, S=2048, E=1024, H=16, D=64, fp32):
    q = rope((x @ Wq.T).heads); k = rope((x @ Wk.T).heads); v = (x @ Wv.T).heads
    out = softmax(mask(q k^T / sqrt(E))) v  -> concat heads -> @ Wo.T

Sharding (8 NeuronCores): data parallel over B (2 groups of 4 cores),
tensor parallel over heads within each group (4 heads per core).
Each core computes QKV for its 4 heads, flash-style causal attention,
then a ROW-PARALLEL Wo partial product: out_partial = attn_own @ Wo.T[own,:]
giving a full-width (S, E) bf16 partial per core.  The host sums the 4
partials per batch group (the unshard step) -- no device collective.

v2 schedule notes:
  - Input DMA order is arranged around the critical path to first
    attention chunk: wq/wk/x j-interleaved first (the QK projection for
    chunk 0 consumes block j as it lands), cos/sin mid-stream, then wv
    (V projections tolerate the later arrival), then wo (needed ~40us in).
  - The 12 non-upfront QK projection groups carry one V-projection
    sequence block each, j-interleaved with the QK matmuls so the V
    LDWEIGHTS (xt block) hides under the QK N=512 streams.
  - Scores are issued two k-blocks ahead of PV so the PE never stalls on
    the exp.  Diagonal k-blocks are tail-sliced: only the causally-valid
    column tail is exp'd / masked / PV'd.
"""

import os
import sys

sys.path.insert(0, "/opt/trn_rl_repo")

import numpy as np
import ml_dtypes

import concourse.bass as bass
import concourse.bacc as bacc
import concourse.mybir as mybir
import concourse.tile as tile
from concourse import bass_utils

B, S, E, H, D = 2, 2048, 1024, 16, 64
NCORES = 8
TP = 4                 # tensor-parallel group size
HPC = H // TP          # heads per core = 4
DQ = HPC * D           # per-core projection width = 256
ATTN_SCALE = 1.0 / float(np.sqrt(E))

FP32 = mybir.dt.float32
BF16 = mybir.dt.bfloat16

SQT = 512              # query chunk width
SKB = 128              # key block (partition dim of score tiles)
NSQT = S // SQT        # 4
NST16 = S // 128       # 16
NE = E // 128          # 8 contraction steps

_CACHE = {}
LAST_RESULT = None


def build_nc():
    nc = bacc.Bacc(None, target_bir_lowering=False)

    xT = nc.declare_dram_parameter("xT", [E, S], BF16, isOutput=False)
    wqT = nc.declare_dram_parameter("wqT", [E, DQ], BF16, isOutput=False)
    wkT = nc.declare_dram_parameter("wkT", [E, DQ], BF16, isOutput=False)
    wvT = nc.declare_dram_parameter("wvT", [E, DQ], BF16, isOutput=False)
    woT = nc.declare_dram_parameter("woT", [DQ, E], BF16, isOutput=False)
    cosd = nc.declare_dram_parameter("cos", [128, S], BF16, isOutput=False)
    sind = nc.declare_dram_parameter("sin", [128, S], BF16, isOutput=False)
    out_ext = nc.declare_dram_parameter("out", [S, E], BF16, isOutput=True)

    with tile.TileContext(nc) as tc:
        with tc.tile_pool(name="const", bufs=1) as constpool:
            # ---- persistent SBUF tensors; DMA order gates pipeline start ----
            w_sb = {}
            for name in ("wq", "wk", "wv"):
                w_sb[name] = constpool.tile(
                    [128, NE * DQ], BF16, tag=f"w_{name}", name=f"w_{name}"
                )
            # wo: 2 contraction blocks (128 own-dims each) x full E columns
            wo_sb = constpool.tile([128, 2 * E], BF16, tag="w_wo", name="w_wo")

            cos_sb = constpool.tile([128, S], BF16, tag="cos")
            sin_sb = constpool.tile([128, S], BF16, tag="sin")

            xt = [
                constpool.tile([128, S], BF16, tag=f"xT{j}", name=f"xT{j}")
                for j in range(NE)
            ]

            qt_sb = [
                constpool.tile([128, S], BF16, tag=f"qt{g}", name=f"qt{g}")
                for g in range(2)
            ]
            kt_sb = [
                constpool.tile([128, S], BF16, tag=f"kt{g}", name=f"kt{g}")
                for g in range(2)
            ]
            vaug = [
                constpool.tile([128, HPC * 65], BF16, tag=f"vaug{i}", name=f"vaug{i}")
                for i in range(NST16)
            ]
            # attention outputs (normalized), head-pair stacked: attnT[j]
            # holds heads 2j (rows 0-63) and 2j+1 (rows 64-127), (d, s).
            attnT = [
                constpool.tile([128, S], BF16, tag=f"attn{j}", name=f"attn{j}")
                for j in range(2)
            ]
            # ones row (fp32) for the PE-side 1/l broadcast: a K=1 matmul
            # ones1.T @ linv (a [1,SQT] reciprocal row) replicates linv
            # onto 64 output partitions.
            ones1 = constpool.tile([1, 64], FP32, tag="ones1", name="ones1")

            # DMA order == critical-path order.  proj(0,*) consumes
            # (wq_j, wk_j, xt_j) per contraction step j, so those three
            # stream j-interleaved.  cos/sin must land before the first
            # rope (~2/3 through the qk stream).  wv gates only the V
            # projections whose outputs are first consumed by PV(0) a
            # couple of microseconds after attention starts.  wo last.
            def qkx_block(j):
                nc.sync.dma_start(
                    out=w_sb["wq"][:, j * DQ:(j + 1) * DQ],
                    in_=wqT[j * 128:(j + 1) * 128, :],
                )
                nc.sync.dma_start(
                    out=w_sb["wk"][:, j * DQ:(j + 1) * DQ],
                    in_=wkT[j * 128:(j + 1) * 128, :],
                )
                nc.sync.dma_start(out=xt[j][:], in_=xT[j * 128:(j + 1) * 128, :])

            for j in range(4):
                qkx_block(j)
            nc.sync.dma_start(out=cos_sb[:], in_=cosd[:])
            nc.sync.dma_start(out=sin_sb[:], in_=sind[:])
            for j in range(4, NE):
                qkx_block(j)
            for j in range(NE):
                nc.sync.dma_start(
                    out=w_sb["wv"][:, j * DQ:(j + 1) * DQ],
                    in_=wvT[j * 128:(j + 1) * 128, :],
                )
            for j in range(2):
                nc.sync.dma_start(
                    out=wo_sb[:, j * E:(j + 1) * E],
                    in_=woT[j * 128:(j + 1) * 128, :],
                )

            # ------- Phase 2: software-pipelined chunk loop -------
            #
            # PSUM budget (8 banks of 2KB/partition):
            #   tag "ps" [128,1024] fp32 x 2 bufs = 4 banks (score tiles,
            #       Wo psw blocks, and V-proj psv accumulators rotate here)
            #   tag "pj" [128,512]  fp32 x 2 bufs = 2 banks (QK proj outs)
            #   tag "po" [128,512]  fp32 x 2 bufs = 2 banks (pso accum)
            with (
                tc.tile_pool(name="pss", bufs=2, space="PSUM") as psspool,
                tc.tile_pool(name="pj", bufs=2, space="PSUM") as pjpool,
                tc.tile_pool(name="po", bufs=2, space="PSUM") as popool,
                tc.tile_pool(name="pt", bufs=6) as ptpool,
                tc.tile_pool(name="rope", bufs=3) as rtpool,
                tc.tile_pool(name="fin", bufs=2) as finpool,
                tc.tile_pool(name="osb", bufs=6) as osbpool,
            ):

                def memset_batch():
                    # one gpsimd library load for ALL memsets; during the
                    # attention loops gpsimd then runs only affine_select
                    # (no ucode-lib thrash, which cost multi-us stalls).
                    nc.gpsimd.memset(ones1[:], 1.0)
                    for i in range(NST16):
                        nc.gpsimd.memset(vaug[i][:], 1.0)

                def rope_tail(st, g, wname, ps, dst):
                    """ACT copy + DVE rotate-half/mul/add after a QK group."""
                    sq0 = st * SQT
                    sq = slice(sq0, sq0 + SQT)
                    raw = rtpool.tile([128, SQT], BF16, tag="raw",
                                      name=f"raw_{wname}_{g}_{st}")
                    nc.scalar.copy(raw[:], ps[:])
                    sw_b = rtpool.tile([128, SQT], BF16, tag="swb",
                                       name=f"swb_{wname}_{g}_{st}")
                    for half in range(2):
                        b0 = half * 64
                        nc.vector.tensor_copy(
                            sw_b[b0:b0 + 32, :], raw[b0 + 32:b0 + 64, :]
                        )
                        nc.vector.tensor_copy(
                            sw_b[b0 + 32:b0 + 64, :], raw[b0:b0 + 32, :]
                        )
                    t1 = rtpool.tile([128, SQT], BF16, tag="t1")
                    nc.vector.tensor_mul(t1[:], sw_b[:], sin_sb[:, sq])
                    t2 = rtpool.tile([128, SQT], BF16, tag="t2")
                    nc.vector.tensor_mul(t2[:], raw[:], cos_sb[:, sq])
                    nc.vector.tensor_add(dst[g][:, sq], t1[:], t2[:])

                def v_finish(i, psv):
                    """strided copy dropping V into vaug[i] (col 64 of each
                    65-slot stays 1.0 from the upfront memset)."""
                    nc.scalar.copy(
                        vaug[i][:, 0:HPC * 65].rearrange(
                            "p (h w) -> p h w", h=HPC
                        )[:, :, 0:64],
                        psv.rearrange("p (h w) -> p h w", h=HPC),
                    )

                def proj_group(st, g, wname, dst):
                    """QK projection group (dense PE matmuls) + rope."""
                    sq0 = st * SQT
                    sq = slice(sq0, sq0 + SQT)
                    ps = pjpool.tile([128, SQT], FP32, tag="pj",
                                     name=f"pj_{wname}_{g}_{st}")
                    for j in range(NE):
                        nc.tensor.matmul(
                            ps[:],
                            lhsT=w_sb[wname][
                                :, j * DQ + g * 128: j * DQ + g * 128 + 128
                            ],
                            rhs=xt[j][:, sq],
                            start=(j == 0),
                            stop=(j == NE - 1),
                        )
                    rope_tail(st, g, wname, ps[:], dst)

                def v_group(i):
                    """Standalone V projection for sequence block i (upfront
                    only; psv rides a 'ps'-tag PSUM slot)."""
                    psv = pjpool.tile([128, DQ], FP32, tag="pj",
                                      name=f"psv{i}")[:]
                    for j in range(NE):
                        nc.tensor.matmul(
                            psv,
                            lhsT=xt[j][:, i * 128:(i + 1) * 128],
                            rhs=w_sb["wv"][:, j * DQ:(j + 1) * DQ],
                            start=(j == 0),
                            stop=(j == NE - 1),
                        )
                    v_finish(i, psv)

                def proj_v_group(st, g, wname, dst, vi):
                    """Fused QK projection + V projection for sequence
                    block vi, j-interleaved so the V LDWEIGHTS (xt block)
                    hides under the QK N=512 streams."""
                    sq0 = st * SQT
                    sq = slice(sq0, sq0 + SQT)
                    ps = pjpool.tile([128, SQT], FP32, tag="pj",
                                     name=f"pj_{wname}_{g}_{st}")
                    psv = pjpool.tile([128, DQ], FP32, tag="pj",
                                      name=f"psv{vi}")[:]
                    for j in range(NE):
                        nc.tensor.matmul(
                            ps[:],
                            lhsT=w_sb[wname][
                                :, j * DQ + g * 128: j * DQ + g * 128 + 128
                            ],
                            rhs=xt[j][:, sq],
                            start=(j == 0),
                            stop=(j == NE - 1),
                        )
                        nc.tensor.matmul(
                            psv,
                            lhsT=xt[j][:, vi * 128:(vi + 1) * 128],
                            rhs=w_sb["wv"][:, j * DQ:(j + 1) * DQ],
                            start=(j == 0),
                            stop=(j == NE - 1),
                        )
                    rope_tail(st, g, wname, ps[:], dst)
                    v_finish(vi, psv)

                def wo_start(st, i4):
                    """First contraction half (head pair 0) of a Wo block;
                    depends only on attnT[0], so it can pre-run during the
                    final head pair's finalize chain to keep the PE warm."""
                    r0 = st * SQT + i4 * 128
                    osb = osbpool.tile([128, E], BF16, tag="osb",
                                       name=f"osb{r0}")
                    psw = psspool.tile([128, 2 * SQT], FP32, tag="ps",
                                       name=f"psw_{r0}")
                    for nh in range(2):
                        nc.tensor.matmul(
                            psw[:, nh * SQT:(nh + 1) * SQT],
                            lhsT=attnT[0][:, r0:r0 + 128],
                            rhs=wo_sb[:, nh * 512: nh * 512 + 512],
                            start=True,
                            stop=False,
                        )
                    return r0, osb, psw

                def wo_finish(r0, osb, psw, tail):
                    for nh in range(2):
                        nc.tensor.matmul(
                            psw[:, nh * SQT:(nh + 1) * SQT],
                            lhsT=attnT[1][:, r0:r0 + 128],
                            rhs=wo_sb[:, E + nh * 512: E + nh * 512 + 512],
                            start=False,
                            stop=True,
                        )
                    if tail:
                        # exps are done: split the evacuation ACT/DVE and
                        # ship each half as soon as it lands
                        nc.scalar.copy(osb[:, 0:SQT], psw[:, 0:SQT])
                        nc.sync.dma_start(
                            out=out_ext[r0:r0 + 128, 0:SQT], in_=osb[:, 0:SQT]
                        )
                        nc.vector.tensor_copy(
                            osb[:, SQT:2 * SQT], psw[:, SQT:2 * SQT]
                        )
                        nc.sync.dma_start(
                            out=out_ext[r0:r0 + 128, SQT:2 * SQT],
                            in_=osb[:, SQT:2 * SQT],
                        )
                    else:
                        # DVE only: ScalarE must keep streaming exps
                        nc.vector.tensor_copy(osb[:], psw[:])
                        nc.sync.dma_start(
                            out=out_ext[r0:r0 + 128, :], in_=osb[:]
                        )

                def wo_block(st, i4, tail=False):
                    """One 128-row block of the row-parallel Wo partial."""
                    r0, osb, psw = wo_start(st, i4)
                    wo_finish(r0, osb, psw, tail)

                def attn_loop(st, g, inserts):
                    """Causal attention for (chunk st, head pair g).
                    `inserts` is a list of (kb_pos, closure) fired right
                    after that k-block's PV, soaking spare PE cycles under
                    the ACT-bound exp stream."""
                    sq0 = st * SQT
                    sq = slice(sq0, sq0 + SQT)
                    nblk = (sq0 + SQT) // SKB
                    pso = [
                        popool.tile([128, SQT], FP32, tag="po",
                                    name=f"pso{p}_{g}_{st}")
                        for p in range(2)
                    ]

                    def issue_scores(kb):
                        c0 = max(0, kb * SKB - sq0)
                        ps = psspool.tile(
                            [128, 2 * SQT], FP32, tag="ps",
                            name=f"pss_{g}_{st}_{kb}",
                        )
                        for p in range(2):
                            nc.tensor.matmul(
                                ps[:, p * SQT + c0:(p + 1) * SQT],
                                lhsT=kt_sb[g][
                                    p * 64:(p + 1) * 64,
                                    kb * SKB:(kb + 1) * SKB,
                                ],
                                rhs=qt_sb[g][
                                    p * 64:(p + 1) * 64, sq0 + c0:sq0 + SQT
                                ],
                                start=True,
                                stop=True,
                            )
                        return ps, c0

                    pt_store = {}

                    def exp_part(kb, ps, c0):
                        w = SQT - c0
                        pt = ptpool.tile([128, 2 * SQT], BF16, tag="pt",
                                         name=f"pt_{g}_{st}_{kb}")
                        if c0 == 0:
                            nc.scalar.activation(
                                pt[:], ps[:],
                                mybir.ActivationFunctionType.Exp,
                                scale=ATTN_SCALE,
                            )
                        else:
                            psview = ps[:].rearrange(
                                "p (h w) -> p h w", h=2)[:, :, c0:]
                            ptview = pt[:].rearrange(
                                "p (h w) -> p h w", h=2)[:, :, c0:]
                            nc.scalar.activation(
                                ptview, psview,
                                mybir.ActivationFunctionType.Exp,
                                scale=ATTN_SCALE,
                            )
                        if kb * SKB >= sq0:  # diagonal block: mask the tail
                            ptview = pt[:].rearrange(
                                "p (h w) -> p h w", h=2)[:, :, c0:]
                            nc.gpsimd.affine_select(
                                out=ptview,
                                in_=ptview,
                                compare_op=mybir.AluOpType.is_ge,
                                fill=0.0,
                                base=0,
                                channel_multiplier=-1,
                                pattern=[[0, 2], [1, w]],
                            )
                        pt_store[kb] = (pt, c0)

                    def pv_part(kb):
                        pt, c0 = pt_store.pop(kb)
                        for p in range(2):
                            h = 2 * g + p
                            nc.tensor.matmul(
                                pso[p][0:65, c0:SQT],
                                lhsT=vaug[kb][:, h * 65:(h + 1) * 65],
                                rhs=pt[:, p * SQT + c0:(p + 1) * SQT],
                                start=(kb == 0),
                                stop=(kb == nblk - 1),
                            )

                    # 2-kb slots: [exp(k), exp(k+1) | scores(k+2), (k+3) |
                    # PV(k-2), PV(k-1) | inserts].  PV lags the exp stream
                    # by one slot (the pt pool holds the backlog), so PE
                    # work in a slot never waits on that slot's exps, and
                    # same-PE-tile-mode matmuls stay adjacent (one
                    # 64-row/128-row reconfig per direction per slot
                    # instead of two per kb).
                    todo = sorted(inserts, key=lambda x: x[0])
                    pending = {0: issue_scores(0), 1: issue_scores(1)}
                    while todo and todo[0][0] < 0:
                        todo.pop(0)[1]()
                    for kp in range(0, nblk, 2):
                        pe0, c00 = pending.pop(kp)
                        exp_part(kp, pe0, c00)
                        pe1, c01 = pending.pop(kp + 1)
                        exp_part(kp + 1, pe1, c01)
                        # PVs first: their pts are a slot old, so they run
                        # while this slot's exps stream; the score quad
                        # (which must wait for exp(kp) to free its PSUM
                        # slot) follows and can't head-of-line block them.
                        if kp - 2 >= 0:
                            pv_part(kp - 2)
                            pv_part(kp - 1)
                        if kp + 2 < nblk:
                            pending[kp + 2] = issue_scores(kp + 2)
                            pending[kp + 3] = issue_scores(kp + 3)
                        while todo and todo[0][0] <= kp + 1:
                            todo.pop(0)[1]()
                    pv_part(nblk - 2)
                    pv_part(nblk - 1)
                    while todo:
                        todo.pop(0)[1]()

                    # finalize this head pair: 1/l via fast reciprocal,
                    # then a K=1 PE matmul against a ones row broadcasts
                    # linv onto 64 output partitions (keeps gpsimd free of
                    # partition_broadcast and its ucode-library swaps).
                    # Returned as a closure and fired at position -1 of the
                    # NEXT attention loop: the psb matmul then sits behind
                    # that loop's score matmuls in the PE FIFO, so the PE
                    # never head-of-line blocks on this DVE chain.  (-1 is
                    # also required for correctness: it must precede the
                    # next loop's PV(0), which recycles the pso slots.)
                    def fin():
                        lbcs = []
                        for p in range(2):
                            lrow = finpool.tile([1, SQT], FP32,
                                                tag=f"lrow{g}{p}")
                            nc.vector.tensor_copy(lrow[:], pso[p][64:65, :])
                            linv = finpool.tile([1, SQT], FP32,
                                                tag=f"linv{g}{p}")
                            nc.vector.reciprocal_approx_fast(linv[:], lrow[:])
                            lbc_p = finpool.tile([64, SQT], FP32,
                                                 tag=f"lbc{g}{p}")
                            nc.gpsimd.partition_broadcast(lbc_p[:], linv[:])
                            lbcs.append(lbc_p)
                        if st == NSQT - 1 and g == 1:
                            # last chunk: pre-run the attnT[0] half of the
                            # first tail Wo blocks NOW (keeps the PE warm
                            # under this serial finalize chain), normalize
                            # in column halves, ship blocks as halves land
                            pre = [wo_start(st, 0), wo_start(st, 1)]
                            for half in range(2):
                                cl = slice(half * 256, half * 256 + 256)
                                gcl = slice(sq0 + half * 256,
                                            sq0 + half * 256 + 256)
                                for p in range(2):
                                    nc.vector.tensor_mul(
                                        attnT[g][p * 64:(p + 1) * 64, gcl],
                                        pso[p][0:64, cl],
                                        lbcs[p][:, cl],
                                    )
                                if half == 0:
                                    wo_finish(*pre[0], tail=True)
                                    wo_finish(*pre[1], tail=True)
                                else:
                                    wo_block(st, 2, tail=True)
                                    wo_block(st, 3, tail=True)
                        else:
                            for p in range(2):
                                nc.vector.tensor_mul(
                                    attnT[g][p * 64:(p + 1) * 64, sq],
                                    pso[p][0:64, :],
                                    lbcs[p][:],
                                )

                    return fin

                PJV = lambda s, g, w, vi: (lambda: proj_v_group(
                    s, g, w, qt_sb if w == "wq" else kt_sb, vi))
                WO = lambda s, i: (lambda: wo_block(s, i))

                # Upfront: chunk-0 AND chunk-1 QK projection groups, all
                # pure qk -- their matmuls ride the wq/wk/x DMA arrivals,
                # and having proj(1) done before attn(0) ends removes the
                # rope-chain stall in front of attn(1,0).  V(0..7) ride as
                # attn(0,*) inserts: their matmuls wait on the late wv DMA
                # without blocking the score/exp stream.
                V = lambda i: (lambda: v_group(i))
                memset_batch()
                # preload the gpsimd 'attn' ucode library (partition_broadcast
                # lives there; the load DMA costs ~6us) and the ScalarE exp
                # table set (~2.7us) while the input DMAs stream, so neither
                # load lands on the first finalize / first exp.
                warmb = finpool.tile([8, 16], FP32, tag="warmb")
                nc.gpsimd.partition_broadcast(warmb[:], ones1[0:1, 0:16])
                warme = finpool.tile([1, 16], BF16, tag="warme")
                nc.scalar.activation(warme[:], ones1[0:1, 0:16],
                                     mybir.ActivationFunctionType.Exp)
                PJ = lambda st_, g_, w_: (lambda: proj_group(
                    st_, g_, w_, qt_sb if w_ == "wq" else kt_sb))
                proj_group(0, 0, "wq", qt_sb)
                proj_group(0, 0, "wk", kt_sb)

                # Hosting plan: proj(X, pair1) rides in attn(X, pair0);
                # proj(X+1, pair0) rides in attn(X, pair1).  Each hosted
                # projection group carries one V sequence block.  Wo(X)
                # rides in the chunk-X+1 loops after finalize(X) drains.
                ins = {
                    (0, 0): [(-1, PJ(0, 1, "wq")), (-1, V(0)),
                             (0, PJ(0, 1, "wk")), (0, V(1)),
                             (1, V(2)), (1, V(3))],
                    (0, 1): [(-1, PJ(1, 0, "wq")), (0, PJ(1, 0, "wk")),
                             (1, V(4)), (1, V(5)), (2, V(6)), (2, V(7))],
                    (1, 0): [(-1, PJV(1, 1, "wq", 8)), (2, PJV(1, 1, "wk", 9)),
                             (4, WO(0, 0)), (5, WO(0, 1))],
                    (1, 1): [(-1, PJV(2, 0, "wq", 10)), (2, PJV(2, 0, "wk", 11)),
                             (4, WO(0, 2)), (5, WO(0, 3))],
                    (2, 0): [(-1, PJV(2, 1, "wq", 12)), (2, PJV(2, 1, "wk", 13)),
                             (5, WO(1, 0)), (7, WO(1, 1)), (9, WO(1, 2))],
                    (2, 1): [(-1, PJV(3, 0, "wq", 14)), (2, PJV(3, 0, "wk", 15)),
                             (5, WO(1, 3))],
                    (3, 0): [(-1, PJ(3, 1, "wq")), (2, PJ(3, 1, "wk")),
                             (4, WO(2, 0)), (7, WO(2, 1)), (10, WO(2, 2)),
                             (13, WO(2, 3))],
                    (3, 1): [],
                }
                pending_fin = None
                for st in range(NSQT):
                    for g in range(2):
                        extra = ([(-1, pending_fin)] if pending_fin else [])
                        pending_fin = attn_loop(st, g,
                                                extra + ins[(st, g)])
                pending_fin()

    nc.finalize()
    return nc


def _host_tables():
    inv = 1.0 / (10000.0 ** (np.arange(0, D, 2, dtype=np.float64) / D))  # (32,)
    ang = np.arange(S, dtype=np.float64)[None, :] * inv[:, None]          # (32,S)
    cos32 = np.cos(ang)
    sin32 = np.sin(ang)
    cos = np.tile(cos32, (4, 1)).astype(np.float32)                       # (128,S)
    sin = np.concatenate([-sin32, sin32, -sin32, sin32], axis=0).astype(np.float32)
    return cos, sin


def kernel(x, W_q, W_k, W_v, W_o):
    global LAST_RESULT
    if "nc" not in _CACHE:
        _CACHE["nc"] = build_nc()
    nc = _CACHE["nc"]

    bf = ml_dtypes.bfloat16
    perm = np.concatenate([np.arange(0, D, 2), np.arange(1, D, 2)])
    rowperm = (np.arange(H)[:, None] * D + perm[None, :]).reshape(-1)
    Wq_p = W_q[rowperm]
    Wk_p = W_k[rowperm]
    cos, sin = _host_tables()

    in_maps = []
    for c in range(NCORES):
        b, tp = c // TP, c % TP
        sl = slice(tp * DQ, (tp + 1) * DQ)
        in_maps.append({
            "xT": np.ascontiguousarray(x[b].T).astype(bf),
            "wqT": np.ascontiguousarray(Wq_p[sl].T).astype(bf),
            "wkT": np.ascontiguousarray(Wk_p[sl].T).astype(bf),
            "wvT": np.ascontiguousarray(W_v[sl].T).astype(bf),
            # row-parallel Wo: rows of Wo.T for this core's attn dims
            "woT": np.ascontiguousarray(W_o[:, sl].T).astype(bf),
            "cos": cos.astype(bf),
            "sin": sin.astype(bf),
        })

    res = bass_utils.run_bass_kernel_spmd(
        nc, in_maps, core_ids=list(range(NCORES)),
        tmpdir=os.environ.get("BASS_TMPDIR") or None,
    )
    LAST_RESULT = res
    out = np.zeros((B, S, E), np.float32)
    for c in range(NCORES):
        b = c // TP
        out[b] += np.asarray(res.results[c]["out"], dtype=np.float32)
    return out
